# revision 40
# baseline (speedup 1.0000x reference)
"""Trainium2 Bass kernel for the augmented neural-ODE RK4(3/8) solver.

Integrates y' = f(y) = W2 @ tanh(W1 @ y + b1) + b2 with the RK4 3/8 rule
over the time grid, data-parallel over the batch axis across 8 NeuronCores.

Per-core layout: state y is [D=128 partitions, 768 tokens] (tokens =
3 traj x 256 batch-shard).  Both MLP matmuls contract over the partition
axis with stationary weights; tanh runs on ScalarE with the per-partition
bias fused; the RK4 linear combinations use fused scalar_tensor_tensor
(out = (in0 * c) op in1) ops, one per k-term:

    u2 = y + (h/3) q1
    t  = y - (h/3) q1
    u3 = t + h q2
    w  = 2 u2 - u3                (= y + h q1 - h q2)
    u4 = w + h q3
    z1 = 2 u3 + u4
    z2 = z1 - y/3
    y' = (3/8) z2 + (h/8) q4      (= y + h(q1 + 3 q2 + 3 q3 + q4)/8)

Matmul operands use float32r (fp32 with 11-bit mantissa, 1 PE cycle/column
vs 4 for fp32; PSUM accumulation stays full fp32).  The integration state
itself is kept in full fp32 — each step makes one rounded fp32r copy of the
state for the tensor engine, so rounding error only enters through the
vector-field evaluations (measured ~1.5e-4 per matmul).

b1 (and any b2 flow-through) is folded into the tanh bias host-side; a
nonzero b2 additionally adds h*b2 to y' via one tensor_scalar op per step
(skipped entirely when b2 == 0, which is the shipped configuration).
"""

import sys

# Fallback only: the container's sitecustomize already provides the repo
# (appended so the pre-imported copy always wins; prepending would mix
# two different repo checkouts).
if "/opt/trn_rl_repo" not in sys.path:
    sys.path.append("/opt/trn_rl_repo")

from contextlib import ExitStack

import numpy as np

import concourse.mybir as mybir
import concourse.tile as tile
from concourse import bacc
from concourse.bass_utils import run_bass_kernel_spmd

NCORES = 8
TRAJ, BATCH, LAT = 3, 2048, 123
D, HID, T = 128, 256, 50
AUG = D - LAT  # 5
BSH = BATCH // NCORES  # 256
TOK = TRAJ * BSH  # 768
NSTEP = T - 1  # 49

F32 = mybir.dt.float32
F32R = mybir.dt.float32r
Alu = mybir.AluOpType
ActF = mybir.ActivationFunctionType

# --- tuning knobs -----------------------------------------------------------
CH = 384  # token chunk for the general-bias path (psum tile = 1 bank)
CHUNKS = [(0, CH), (CH, CH)]
# fast (zero-bias) path: small chunk first so it leads the recurrence
CHUNKS_F = [(0, 256), (256, 512)]
P_BUFS = 4
Q_BUFS = 4
H_BUFS = 16
WORK_BUFS = 2
USE_F32R = True  # float32r matmul operands (4x PE throughput)
# engine for the SBUF-only wide combine ops: "vector" or "gpsimd"
WIDE_ENGINE = "vector"


def round_f32r(x):
    """Round fp32 -> fp32r (11-bit mantissa) with round-to-nearest-even,
    matching the hardware's downconv (verified bit-exact vs TRN2)."""
    u = np.ascontiguousarray(x, dtype=np.float32).view(np.uint32).copy()
    u += 0x7FF + ((u >> 12) & 1)
    u &= np.uint32(0xFFFFF000)
    return u.view(np.float32)


def _build_program(hs, n_bias_sets, has_b2corr, nstep, fast=False):
    """Trace + compile the per-core Bass program.  hs: per-step dt floats.

    fast=True (requires b1 == 0, b2 == 0 and a uniform time grid, which is
    the shipped configuration) takes the fast path:
      - both hidden blocks of the layer-1 pre-activation live side by side
        in one PSUM tile (same partitions, different free offsets), so tanh
        is a single ACT op per token chunk;
      - y' - y is accumulated directly in PSUM by the tensor engine using
        host-prescaled W2 variants ((h/8)W2 and (3h/8)W2), eliminating the
        z-chain vector ops;
      - three equal 256-token chunks keep each chunk's serial recurrence
        (l1 -> tanh -> l2 -> combine) short and pipelined.
    """
    MMT = F32R if USE_F32R else F32
    nc = bacc.Bacc(
        "TRN2", target_bir_lowering=False, debug=False, enable_asserts=False
    )
    y0_t = nc.dram_tensor("y0", [D, TOK], F32, kind="ExternalInput").ap()
    # All PE weights live in ONE packed tensor loaded by ONE DMA, so a single
    # warm-up matmul makes the PE observe the whole weight load (fused-weight
    # matmuls may carry only one sync wait).
    # 128-col weight blocks: w1t(2) w2t(2) | fast adds prescaled W2 variants:
    # w1t(2) (h/3)W2(2) -(h/3)W2(2) hW2(2) -hW2(2) (h/8)W2(2) (3h/8)W2(2)
    NW = 14 if fast else 4
    wpack_t = nc.dram_tensor("wpack", [D, NW * D], MMT, kind="ExternalInput").ap()
    bias_t = nc.dram_tensor(
        "biases", [D, n_bias_sets * 8], F32, kind="ExternalInput"
    ).ap()
    if has_b2corr:
        b2h_t = nc.dram_tensor("b2h", [D, nstep], F32, kind="ExternalInput").ap()
    out_t = nc.dram_tensor("out", [nstep, D, TOK], F32, kind="ExternalOutput").ap()

    with tile.TileContext(nc) as tc, ExitStack() as ctx:
        cpool = ctx.enter_context(tc.tile_pool(name="consts", bufs=1))
        wpool = ctx.enter_context(tc.tile_pool(name="work", bufs=WORK_BUFS))
        hpool = ctx.enter_context(tc.tile_pool(name="hid", bufs=H_BUFS))
        # PSUM budget is 8 banks.  Fast path: p tiles are [D, 1024] (2 banks,
        # one per hidden block so the two layer-1 matmuls never share a
        # bank) x 2 bufs = 4 banks; q/yb share one tag x 4 bufs = 4 banks.
        ppool = ctx.enter_context(
            tc.tile_pool(name="pp", bufs=(2 if fast else P_BUFS), space="PSUM")
        )
        qpool = ctx.enter_context(
            tc.tile_pool(name="qp", bufs=(4 if fast else Q_BUFS), space="PSUM")
        )

        wide = nc.gpsimd if WIDE_ENGINE == "gpsimd" else nc.vector

        wpack = cpool.tile([D, NW * D], MMT, name="wpack_sb")
        nc.sync.dma_start(wpack[:], wpack_t[:])
        wblk = [wpack[:, k * D : (k + 1) * D] for k in range(NW)]
        w1t = wpack[:, 0 : 2 * D]  # [D, 256]
        w2t = [wblk[2], wblk[3]]
        biases = cpool.tile([D, n_bias_sets * 8], F32, name="bias_sb")
        nc.sync.dma_start(biases[:], bias_t[:])
        if fast:
            w2_3 = [wblk[2], wblk[3]]    # (h/3) W2^T
            w2_3n = [wblk[4], wblk[5]]   # -(h/3) W2^T
            w2h = [wblk[6], wblk[7]]     # h W2^T
            w2hn = [wblk[8], wblk[9]]    # -h W2^T
            w2e = [wblk[10], wblk[11]]   # (h/8) W2^T
            w2e3 = [wblk[12], wblk[13]]  # (3h/8) W2^T
        if has_b2corr:
            b2h = cpool.tile([D, nstep], F32, name="b2h_sb")
            nc.sync.dma_start(b2h[:], b2h_t[:])

        y = wpool.tile([D, TOK], F32, tag="y", name="y_init")
        nc.sync.dma_start(y[:], y0_t[:])

        # Single warm-up matmul: PE observes the packed-weight DMA once, so
        # no real (fused-weight-load) matmul ever needs a second sync wait —
        # walrus's S3_LW struct only supports one.
        dummy = ppool.tile([D, 8], F32, tag="p", name="warm")
        nc.tensor.matmul(
            dummy[:], wpack[:, 0:D], wpack[:, 0:8], start=True, stop=True
        )

        if fast:
            CW = 256
            fchunks = [(0, CW), (CW, CW), (2 * CW, CW)]
            h0 = float(hs[0])
            for n in range(nstep):
                y_r = wpool.tile([D, TOK], F32R, tag="yr", name="yr")
                for off, cw in fchunks:
                    nc.vector.tensor_copy(
                        y_r[:, off : off + cw], y[:, off : off + cw]
                    )
                u_in = y_r
                hsave = [[None] * 3 for _ in range(4)]
                # Every combine is u = (PSUM bank) + y, so each vector op and
                # each matmul carries at most ONE un-observed semaphore wait
                # (walrus compute instructions only support one).  Bank
                # contribution lists per eval, in terms of saved h_i:
                #   ev0 (u2-y):  (h/3)W2 h1
                #   ev1 (u3-y):  h W2 h2 - (h/3)W2 h1
                #   ev2 (u4-y):  h W2 h1 - h W2 h2 + h W2 h3
                #   ev3 (y'-y):  (h/8)W2 (h1+h4) + (3h/8)W2 (h2+h3)
                for ev in range(4):
                    un = wpool.tile(
                        [D, TOK + 8 if ev == 3 else TOK],
                        F32 if ev == 3 else F32R,
                        tag=("y" if ev == 3 else f"u{ev}"),
                        name=("yn" if ev == 3 else f"u{ev}"),
                    )
                    if ev == 3:
                        # Sacrificial first writer in the pad column (outside
                        # every chunk range so the combines don't WAW on it):
                        # absorbs the slot-release waits, incl. the output-DMA
                        # read of the tile two steps ago.
                        nc.gpsimd.memset(un[:, TOK : TOK + 1], 0.0)
                    for ci, (off, cw) in enumerate(fchunks):
                        # [D, 1024]: hidden block 0 at cols 0:cw (bank 0),
                        # block 1 at cols 512:512+cw (bank 1) — separate
                        # banks so the two matmuls aren't serialized by the
                        # same-bank tracker.
                        p = ppool.tile([D, 1024], F32, tag="p", name=f"p{ci}")
                        # Sacrificial first writer: absorbs the slot-release
                        # semaphore wait so the real matmuls below carry only
                        # their data dependency.
                        nc.tensor.matmul(
                            p[:, 1016:1024], wpack[:, 0:D], wpack[:, 0:8],
                            start=True, stop=True,
                        )
                        for hb in (0, 1):
                            nc.tensor.matmul(
                                p[:, hb * 512 : hb * 512 + cw],
                                w1t[:, hb * D : (hb + 1) * D],
                                u_in[:, off : off + cw],
                                start=True,
                                stop=True,
                            )
                        ht = hpool.tile(
                            [D, 2 * cw + 8], MMT, tag="h", name=f"h{ci}"
                        )
                        # Sacrificial first writer in the pad column (GPSIMD:
                        # not a TPB-struct op, so it may carry several waits):
                        # absorbs the h-slot release so tanh carries only its
                        # PE data dependency.
                        nc.gpsimd.memset(ht[:, 2 * cw : 2 * cw + 1], 0.0)
                        p_view = p[:].rearrange("p (s c) -> p s c", s=2)[:, :, 0:cw]
                        h_view = ht[:, 0 : 2 * cw].rearrange(
                            "p (s c) -> p s c", s=2
                        )
                        nc.scalar.activation(h_view, p_view, ActF.Tanh)
                        hsave[ev][ci] = ht
                        sl = slice(off, off + cw)
                        # accumulate this eval's bank from saved h's; the
                        # fresh-h contribution goes LAST so the earlier
                        # matmuls run during its tanh
                        contribs = (
                            ((ht, w2_3),),
                            ((hsave[0][ci], w2_3n), (ht, w2h)),
                            (
                                (hsave[0][ci], w2h),
                                (hsave[1][ci], w2hn),
                                (ht, w2h),
                            ),
                            (
                                (hsave[0][ci], w2e),
                                (hsave[1][ci], w2e3),
                                (hsave[2][ci], w2e3),
                                (ht, w2e),
                            ),
                        )[ev]
                        bank = qpool.tile([D, cw], F32, tag="q", name=f"b{ci}")
                        for k, (sv, wv) in enumerate(contribs):
                            nc.tensor.matmul(
                                bank[:], wv[0][:], sv[:, 0:cw],
                                start=(k == 0), stop=False,
                            )
                            nc.tensor.matmul(
                                bank[:], wv[1][:], sv[:, cw : 2 * cw],
                                start=False, stop=(k == len(contribs) - 1),
                            )
                        nc.vector.tensor_add(un[:, sl], bank[:], y[:, sl])
                    if ev == 3:
                        nc.sync.dma_start(out_t[n], un[:, 0:TOK])
                        y = un
                    else:
                        u_in = un
            nc.compile()
            return nc

        chunks = CHUNKS

        def emit_l1_tanh(u_in, bias_cols):
            """Layer 1 + tanh; returns {ci: (h0_tile, h1_tile)}."""
            ps = {}
            for hb in (0, 1):
                for ci, (off, cw) in enumerate(chunks):
                    p = ppool.tile([D, cw], F32, tag="p", name=f"p{hb}{ci}")
                    nc.tensor.matmul(
                        p[:],
                        w1t[:, hb * D : (hb + 1) * D],
                        u_in[:, off : off + cw],
                        start=True,
                        stop=True,
                    )
                    ps[hb, ci] = p
            hts = {}
            for ci in range(len(chunks)):
                pair = []
                for hb in (0, 1):
                    cw = chunks[ci][1]
                    ht = hpool.tile([D, cw], MMT, tag="h", name=f"h{hb}{ci}")
                    nc.scalar.activation(
                        ht[:],
                        ps[hb, ci][:],
                        ActF.Tanh,
                        bias=biases[:, bias_cols + hb : bias_cols + hb + 1],
                        scale=1.0,
                    )
                    pair.append(ht)
                hts[ci] = pair
            return hts

        def emit_l2_chunk(hts, ci):
            """Layer 2 for one chunk; returns the q psum tile."""
            cw = chunks[ci][1]
            q = qpool.tile([D, cw], F32, tag=f"q{cw}", name=f"q{ci}", bufs=2)
            h0, h1 = hts[ci]
            nc.tensor.matmul(q[:], w2t[0][:], h0[:], start=True, stop=False)
            nc.tensor.matmul(q[:], w2t[1][:], h1[:], start=False, stop=True)
            return q

        for n in range(nstep):
            h = float(hs[n])
            s = n if n_bias_sets > 1 else 0
            # rounded fp32r view of the state for the tensor engine
            if USE_F32R:
                y_r = wpool.tile([D, TOK], F32R, tag="yr", name="yr")
                for ci, (off, cw) in enumerate(chunks):
                    nc.vector.tensor_copy(
                        y_r[:, off : off + cw], y[:, off : off + cw]
                    )
            else:
                y_r = y
            u_in = y_r
            u2 = t = u3 = w_t = u4 = z3 = None
            for ev in range(4):
                bb = (s * 4 + ev) * 2
                hts = emit_l1_tanh(u_in, bb)
                if ev == 0:
                    u2 = wpool.tile([D, TOK], MMT, tag="u2", name="u2")
                    t = wpool.tile([D, TOK], F32, tag="t", name="t")
                    for ci, (off, cw) in enumerate(chunks):
                        q = emit_l2_chunk(hts, ci)
                        nc.vector.scalar_tensor_tensor(
                            u2[:, off : off + cw], q[:], h / 3.0,
                            y[:, off : off + cw], Alu.mult, Alu.add,
                        )
                        nc.vector.scalar_tensor_tensor(
                            t[:, off : off + cw], q[:], -h / 3.0,
                            y[:, off : off + cw], Alu.mult, Alu.add,
                        )
                    u_in = u2
                elif ev == 1:
                    u3 = wpool.tile([D, TOK], MMT, tag="u3", name="u3")
                    for ci, (off, cw) in enumerate(chunks):
                        q = emit_l2_chunk(hts, ci)
                        nc.vector.scalar_tensor_tensor(
                            u3[:, off : off + cw], q[:], h,
                            t[:, off : off + cw], Alu.mult, Alu.add,
                        )
                    # w = 2 u2 - u3  (SBUF-only)
                    w_t = wpool.tile([D, TOK], F32, tag="w", name="w")
                    wide.scalar_tensor_tensor(
                        w_t[:], u2[:], 2.0, u3[:], Alu.mult, Alu.subtract
                    )
                    u_in = u3
                elif ev == 2:
                    u4 = wpool.tile([D, TOK], MMT, tag="u4", name="u4")
                    for ci, (off, cw) in enumerate(chunks):
                        q = emit_l2_chunk(hts, ci)
                        nc.vector.scalar_tensor_tensor(
                            u4[:, off : off + cw], q[:], h,
                            w_t[:, off : off + cw], Alu.mult, Alu.add,
                        )
                    # z2 = (2 u3 + u4) - y/3 ; z3 = (3/8) z2  (SBUF-only)
                    z1 = wpool.tile([D, TOK], F32, tag="z1", name="z1")
                    wide.scalar_tensor_tensor(
                        z1[:], u3[:], 2.0, u4[:], Alu.mult, Alu.add
                    )
                    z2 = wpool.tile([D, TOK], F32, tag="z2", name="z2")
                    wide.scalar_tensor_tensor(
                        z2[:], y[:], -1.0 / 3.0, z1[:], Alu.mult, Alu.add
                    )
                    z3 = wpool.tile([D, TOK], F32, tag="z3", name="z3")
                    nc.vector.tensor_scalar_mul(z3[:], z2[:], 0.375)
                    u_in = u4
                else:
                    yn = wpool.tile([D, TOK], F32, tag="y", name="yn")
                    for ci, (off, cw) in enumerate(chunks):
                        q = emit_l2_chunk(hts, ci)
                        nc.vector.scalar_tensor_tensor(
                            yn[:, off : off + cw], q[:], h / 8.0,
                            z3[:, off : off + cw], Alu.mult, Alu.add,
                        )
                    if has_b2corr:
                        yc = wpool.tile([D, TOK], F32, tag="yc", name="yc")
                        nc.vector.tensor_scalar_add(
                            yc[:], yn[:], b2h[:, n : n + 1]
                        )
                        yn = yc
                    nc.sync.dma_start(out_t[n], yn[:])
                    y = yn

    nc.compile()
    return nc


def _prepare(first_point, time_steps, W1, b1, W2, b2, nstep):
    first_point = np.asarray(first_point, dtype=np.float32)
    time_steps = np.asarray(time_steps, dtype=np.float32)
    W1 = np.asarray(W1, dtype=np.float32)
    b1 = np.asarray(b1, dtype=np.float32)
    W2 = np.asarray(W2, dtype=np.float32)
    b2 = np.asarray(b2, dtype=np.float32)

    hs = np.diff(time_steps.astype(np.float64)).astype(np.float32)[:nstep]
    has_b2 = bool(np.any(b2))
    h_const = bool(np.allclose(hs, hs[0]))
    n_bias_sets = 1 if (not has_b2 or h_const) else nstep

    # b1_eff[s, ev] = b1 + gamma_ev(h_s) * (W1 @ b2); gamma = [0, h/3, 2h/3, h]
    W1b2 = W1 @ b2  # [HID]
    bias_mat = np.zeros((D, n_bias_sets * 8), dtype=np.float32)
    for si in range(n_bias_sets):
        h = float(hs[si]) if n_bias_sets > 1 else float(hs[0])
        for ev, gam in enumerate((0.0, h / 3.0, 2.0 * h / 3.0, h)):
            be = b1 + gam * W1b2  # [HID]
            for hb in range(2):
                bias_mat[:, (si * 4 + ev) * 2 + hb] = be[hb * D : (hb + 1) * D]

    b2h_mat = None
    if has_b2:
        b2h_mat = np.empty((D, nstep), dtype=np.float32)
        for n in range(nstep):
            b2h_mat[:, n] = float(hs[n]) * b2

    # y0 per core: [D, TOK] with aug channels zero
    y0_full = np.concatenate(
        [first_point, np.zeros((TRAJ, BATCH, AUG), np.float32)], axis=2
    )  # [TRAJ, BATCH, D]
    w1t = np.ascontiguousarray(W1.T)  # [D, HID]
    w2t_raw = W2.T.reshape(2, D, D).transpose(1, 0, 2).reshape(D, 2 * D)
    # w2t_raw: [D, 256] = [Kb0 | Kb1] blocks side by side

    # The fast path (PSUM-bank combines) is kept for reference but disabled:
    # its schedule makes walrus's one-sync-wait-per-TPB-instruction limit
    # fire on some Activation instructions.  The general path compiles
    # cleanly and is within ~25% of it on the cost model.
    FAST_ENABLE = False
    fast = (
        FAST_ENABLE and USE_F32R and not np.any(bias_mat) and not has_b2 and h_const
    )
    h0 = float(hs[0])
    if fast:
        blocks = [
            w1t,
            w2t_raw * (h0 / 3.0),
            w2t_raw * (-h0 / 3.0),
            w2t_raw * h0,
            w2t_raw * (-h0),
            w2t_raw * (h0 / 8.0),
            w2t_raw * (3.0 * h0 / 8.0),
        ]
    else:
        blocks = [w1t, w2t_raw]
    wpack = np.concatenate(blocks, axis=1)  # [D, NW*128]
    if USE_F32R:
        wpack = round_f32r(wpack)

    in_maps = []
    for c in range(NCORES):
        ysh = y0_full[:, c * BSH : (c + 1) * BSH, :]  # [TRAJ, BSH, D]
        ysh = np.ascontiguousarray(ysh.transpose(2, 0, 1).reshape(D, TOK))
        m = {
            "y0": ysh,
            "wpack": wpack,
            "biases": bias_mat,
        }
        if has_b2:
            m["b2h"] = b2h_mat
        in_maps.append(m)
    return hs, n_bias_sets, has_b2, in_maps, y0_full, fast


def _kernel_impl(inputs, nstep=NSTEP, trace=False, tmpdir=None):
    first_point = inputs["first_point"]
    time_steps = inputs["time_steps"]
    assert first_point.shape == (TRAJ, BATCH, LAT), first_point.shape
    hs, n_bias_sets, has_b2, in_maps, y0_full, zero_bias = _prepare(
        first_point,
        time_steps,
        inputs["W1"],
        inputs["b1"],
        inputs["W2"],
        inputs["b2"],
        nstep,
    )
    nc = _build_program(hs, n_bias_sets, has_b2, nstep, fast=zero_bias)
    res = run_bass_kernel_spmd(
        nc, in_maps, core_ids=list(range(NCORES)), trace=trace, tmpdir=tmpdir
    )

    full = np.empty((TRAJ, BATCH, nstep + 1, D), np.float32)
    full[:, :, 0, :] = y0_full
    for c in range(NCORES):
        o = res.results[c]["out"]  # [nstep, D, TOK]
        o = o.reshape(nstep, D, TRAJ, BSH).transpose(2, 3, 0, 1)
        full[:, c * BSH : (c + 1) * BSH, 1:, :] = o
    return full, res


def kernel(**inputs):
    full, _ = _kernel_impl(inputs)
    return full
